# revision 1
# baseline (speedup 1.0000x reference)
"""Trainium2 Bass kernel for dual cross-attention (CotSR) block.

Problem (hardcoded shapes): B=4, C=96, H=W=64 -> N=4096, C8=12, NC=96.
  For each direction d and batch b:
    q = wq @ x_q + bq            [12, N]
    k = wk @ x_kv + bk           [12, N]
    v = wv @ x_kv + bv           [96, N]
    S = q^T k  (S[n, m])         softmax over m (unscaled)
    O = sum_m softmax(S)[n, m] * v[:, m]
    out = wc @ (x_q + gamma * O) + bc

Sharding: 8 independent (direction, batch) units -> 8 NeuronCores.

Per-core device pipeline (all fp32):
  - projections on PE (contraction padded to 128 partitions with zeros)
  - scores computed transposed (S^T[m, n]) so the A@V matmul needs no
    transposes: per n-chunk of 512 query rows, loop m-chunks (512/384
    alternating to ping-pong 4+3 PSUM banks), exp on ScalarE straight out
    of PSUM into SBUF, then accumulate O'[97, n] = [V|1]^T @ E^T into one
    PSUM bank (row 96 = softmax denominator, via the appended ones col).
  - epilogue: r = 1/l broadcast to [96, n] via a rank-1 matmul, normalize,
    then the output conv as two accumulated matmuls (base path carries the
    bias + residual through a ones row; gamma folded into wc host-side).
"""

import numpy as np

B, C, C8, NC = 4, 96, 12, 96
N = 4096  # 64*64
P = 128
NCH = 8  # n chunks of 512
# m-chunk sizes per n-chunk: alternate 4-bank / 3-bank groups (+1 O' bank = 8)
M_SIZES = [512, 384, 512, 384, 512, 384, 512, 384, 512]
assert sum(M_SIZES) == N

_PROG = None


def _split_multi_waits(nc):
    """Walrus in this container rejects >1 sync wait per instruction.
    Split extra waits onto same-engine NOPs inserted just before."""
    import concourse.mybir as mybir

    n_split = 0
    for bb in nc.main_func.blocks:
        insts = list(bb.instructions)
        if not any(i.sync_info and len(i.sync_info.on_wait) > 1 for i in insts):
            continue
        new = []
        for inst in insts:
            si = inst.sync_info
            if si is not None and len(si.on_wait) > 1:
                waits = list(si.on_wait)
                for w in waits[:-1]:
                    n_split += 1
                    new.append(
                        mybir.InstNoOp(
                            name=f"{inst.name}-wsplit{n_split}",
                            engine=inst.engine,
                            ins=[],
                            outs=[],
                            sync_info=mybir.SyncInfo(on_wait=[w], on_update=[]),
                        )
                    )
                inst.sync_info = mybir.SyncInfo(
                    on_wait=[waits[-1]], on_update=list(si.on_update)
                )
            new.append(inst)
        bb.instructions = new
    return n_split


def _build_program():
    import concourse.bass as bass
    import concourse.mybir as mybir
    import concourse.tile as tile

    f32 = mybir.dt.float32
    nc = bass.Bass()

    xq_d = nc.dram_tensor("xq", [P, N], f32, kind="ExternalInput")
    xkv_d = nc.dram_tensor("xkv", [P, N], f32, kind="ExternalInput")
    wqT_d = nc.dram_tensor("wqT", [P, C8], f32, kind="ExternalInput")
    wkT_d = nc.dram_tensor("wkT", [P, C8], f32, kind="ExternalInput")
    wvT_d = nc.dram_tensor("wvT", [P, C], f32, kind="ExternalInput")
    wcA_d = nc.dram_tensor("wcA", [P, NC], f32, kind="ExternalInput")
    wcB_d = nc.dram_tensor("wcB", [P, NC], f32, kind="ExternalInput")
    out_d = nc.dram_tensor("out", [NC, N], f32, kind="ExternalOutput")

    with tile.TileContext(nc) as tc:
        with (
            tc.tile_pool(name="persist", bufs=1) as pp,
            tc.tile_pool(name="dram", bufs=1, space="DRAM") as drp,
        ):
            xq = pp.tile([P, N], f32)
            xkv = pp.tile([P, N], f32)
            wqT = pp.tile([P, C8], f32)
            wkT = pp.tile([P, C8], f32)
            wvT = pp.tile([P, C], f32)
            wcA = pp.tile([P, NC], f32)
            wcB = pp.tile([P, NC], f32)
            nc.sync.dma_start(out=xq, in_=xq_d[:])
            nc.sync.dma_start(out=xkv, in_=xkv_d[:])
            nc.sync.dma_start(out=wqT, in_=wqT_d[:])
            nc.sync.dma_start(out=wkT, in_=wkT_d[:])
            nc.sync.dma_start(out=wvT, in_=wvT_d[:])
            nc.sync.dma_start(out=wcA, in_=wcA_d[:])
            nc.sync.dma_start(out=wcB, in_=wcB_d[:])

            bufQ = pp.tile([P, N], f32)  # rows 0:12 = q, rest zero
            bufK = pp.tile([P, N], f32)  # rows 0:12 = k, rest zero
            VT1 = pp.tile([P, 32 * (C + 1)], f32)  # per m-block [V^T | 1]
            Oun = pp.tile([C + 1, N], f32)  # unnormalized O^T; row 96 = l
            Onorm = pp.tile([P, N], f32)  # rows 0:96 normalized O^T
            rrow = pp.tile([P, N], f32)  # row 0 = 1/l, rest zero
            bcw = pp.tile([P, NC], f32)  # row 0 = 1, rest zero
            l_t = pp.tile([P, 32], f32)
            r_t = pp.tile([P, 32], f32)
            nc.vector.memset(bufQ, 0.0)
            nc.vector.memset(bufK, 0.0)
            nc.vector.memset(VT1, 1.0)
            nc.vector.memset(Onorm, 0.0)
            nc.vector.memset(rrow, 0.0)
            nc.vector.memset(bcw, 0.0)
            nc.vector.memset(bcw[0:1, :], 1.0)

            # ---- projections ----
            with tc.tile_pool(name="ps_setup", bufs=1, space="PSUM") as pss:
                for c in range(NCH):
                    sl = slice(512 * c, 512 * (c + 1))
                    pq = pss.tile([C8, 512], f32, tag="pqk", bufs=2)
                    nc.tensor.matmul(pq, lhsT=wqT, rhs=xq[:, sl], start=True, stop=True)
                    nc.scalar.copy(out=bufQ[0:C8, sl], in_=pq)
                    pk = pss.tile([C8, 512], f32, tag="pqk", bufs=2)
                    nc.tensor.matmul(pk, lhsT=wkT, rhs=xkv[:, sl], start=True, stop=True)
                    nc.scalar.copy(out=bufK[0:C8, sl], in_=pk)
                for mb in range(32):
                    pv = pss.tile([P, C], f32, tag="pv", bufs=4)
                    nc.tensor.matmul(
                        pv,
                        lhsT=xkv[:, 128 * mb : 128 * (mb + 1)],
                        rhs=wvT,
                        start=True,
                        stop=True,
                    )
                    nc.vector.tensor_copy(
                        VT1[:, (C + 1) * mb : (C + 1) * mb + C], pv
                    )

            # ---- main loop: scores -> exp -> A@V ----
            with (
                tc.tile_pool(name="ps_main", bufs=1, space="PSUM") as psm,
                tc.tile_pool(name="epool", bufs=4) as ep,
            ):
                for c in range(NCH):
                    nsl = slice(512 * c, 512 * (c + 1))
                    ps_o = psm.tile([C + 1, 512], f32, tag="ps_o", bufs=1)
                    pending = None  # (e_tile, mb0, nblk)
                    m0 = 0
                    mb0 = 0
                    for t, msz in enumerate(M_SIZES):
                        nblk = msz // 128
                        if t % 2 == 0:
                            sc = psm.tile([P, 2048], f32, tag="sc_even", bufs=1)
                        else:
                            sc = psm.tile([P, 1536], f32, tag="sc_odd", bufs=1)
                        for s in range(nblk):
                            nc.tensor.matmul(
                                sc[:, 512 * s : 512 * (s + 1)],
                                lhsT=bufK[:, m0 + 128 * s : m0 + 128 * (s + 1)],
                                rhs=bufQ[:, nsl],
                                start=True,
                                stop=True,
                            )
                        # software pipeline: issue previous chunk's AV after
                        # this chunk's score matmuls so PE overlaps ACT exp
                        if pending is not None:
                            pe, pmb0, pnblk = pending
                            for s in range(pnblk):
                                mb = pmb0 + s
                                nc.tensor.matmul(
                                    ps_o[0 : C + 1, :],
                                    lhsT=VT1[:, (C + 1) * mb : (C + 1) * (mb + 1)],
                                    rhs=pe[:, 512 * s : 512 * (s + 1)],
                                    start=(mb == 0),
                                    stop=False,
                                )
                        e = ep.tile([P, 2048], f32, tag="e")
                        nc.scalar.activation(
                            out=e[:, : 512 * nblk],
                            in_=sc[:, : 512 * nblk],
                            func=mybir.ActivationFunctionType.Exp,
                        )
                        pending = (e, mb0, nblk)
                        m0 += msz
                        mb0 += nblk
                    pe, pmb0, pnblk = pending
                    for s in range(pnblk):
                        mb = pmb0 + s
                        nc.tensor.matmul(
                            ps_o[0 : C + 1, :],
                            lhsT=VT1[:, (C + 1) * mb : (C + 1) * (mb + 1)],
                            rhs=pe[:, 512 * s : 512 * (s + 1)],
                            start=False,
                            stop=(mb == 31),
                        )
                    nc.vector.tensor_copy(Oun[:, nsl], ps_o[0 : C + 1, :])

            # ---- epilogue ----
            with tc.tile_pool(name="ps_epi", bufs=1, space="PSUM") as pse:
                l_dram = drp.tile([1, N], f32)
                r_dram = drp.tile([1, N], f32)
                with nc.allow_non_contiguous_dma(reason="tiny softmax-denominator reshape"):
                    nc.sync.dma_start(out=l_dram[:], in_=Oun[C : C + 1, :])
                    nc.sync.dma_start(
                        out=l_t, in_=l_dram.rearrange("o (t p) -> (o p) t", p=P)
                    )
                    nc.vector.reciprocal(out=r_t, in_=l_t)
                    nc.sync.dma_start(
                        out=r_dram.rearrange("o (t p) -> (o p) t", p=P), in_=r_t
                    )
                    nc.sync.dma_start(out=rrow[0:1, :], in_=r_dram[:])

                for c in range(NCH):
                    nsl = slice(512 * c, 512 * (c + 1))
                    R = pse.tile([NC, 512], f32, tag="R", bufs=2)
                    nc.tensor.matmul(R, lhsT=bcw, rhs=rrow[:, nsl], start=True, stop=True)
                    nc.vector.tensor_mul(
                        out=Onorm[0:C, nsl], in0=Oun[0:C, nsl], in1=R
                    )
                    pY = pse.tile([NC, 512], f32, tag="pY", bufs=2)
                    nc.tensor.matmul(pY, lhsT=wcA, rhs=xq[:, nsl], start=True, stop=False)
                    nc.tensor.matmul(
                        pY, lhsT=wcB, rhs=Onorm[:, nsl], start=False, stop=True
                    )
                    ybuf = pp.tile([NC, 512], f32, tag="ybuf", bufs=2)
                    nc.vector.tensor_copy(ybuf, pY)
                    nc.sync.dma_start(out=out_d[:, nsl], in_=ybuf)

    _split_multi_waits(nc)
    return nc


def _get_program():
    global _PROG
    if _PROG is None:
        _PROG = _build_program()
    return _PROG


TRACE = False
LAST_RESULT = None


def _pack_weights(wq, bq, wk, bk, wv, bv, gamma, wc, bc):
    wqT = np.zeros((P, C8), np.float32)
    wqT[0:C, :] = wq.T
    wqT[C, :] = bq
    wkT = np.zeros((P, C8), np.float32)
    wkT[0:C, :] = wk.T
    wkT[C, :] = bk
    wvT = np.zeros((P, C), np.float32)
    wvT[0:C, :] = wv.T
    wvT[C, :] = bv
    wcA = np.zeros((P, NC), np.float32)
    wcA[0:C, :] = wc.T
    wcA[C, :] = bc
    wcB = np.zeros((P, NC), np.float32)
    wcB[0:C, :] = float(gamma[0]) * wc.T
    return wqT, wkT, wvT, wcA, wcB


def _pack_x(x):
    buf = np.zeros((P, N), np.float32)
    buf[0:C, :] = x.reshape(C, N)
    buf[C, :] = 1.0
    return buf


def kernel(x1, x2, wq1, bq1, wk1, bk1, wv1, bv1, wq2, bq2, wk2, bk2,
           wv2, bv2, gamma1, gamma2, wc1, bc1, wc2, bc2):
    from concourse.bass_utils import run_bass_kernel_spmd

    global LAST_RESULT
    x1 = np.asarray(x1, np.float32)
    x2 = np.asarray(x2, np.float32)

    w1 = _pack_weights(
        np.asarray(wq1), np.asarray(bq1), np.asarray(wk2), np.asarray(bk2),
        np.asarray(wv2), np.asarray(bv2), np.asarray(gamma1),
        np.asarray(wc1), np.asarray(bc1),
    )
    w2 = _pack_weights(
        np.asarray(wq2), np.asarray(bq2), np.asarray(wk1), np.asarray(bk1),
        np.asarray(wv1), np.asarray(bv1), np.asarray(gamma2),
        np.asarray(wc2), np.asarray(bc2),
    )

    in_maps = []
    for d in range(2):
        xs_q, xs_kv = (x1, x2) if d == 0 else (x2, x1)
        wqT, wkT, wvT, wcA, wcB = w1 if d == 0 else w2
        for b in range(B):
            in_maps.append(
                {
                    "xq": _pack_x(xs_q[b]),
                    "xkv": _pack_x(xs_kv[b]),
                    "wqT": wqT,
                    "wkT": wkT,
                    "wvT": wvT,
                    "wcA": wcA,
                    "wcB": wcB,
                }
            )

    nc = _get_program()
    res = run_bass_kernel_spmd(nc, in_maps, core_ids=list(range(8)), trace=TRACE)
    LAST_RESULT = res

    out1 = np.stack([res.results[b]["out"].reshape(C, 64, 64) for b in range(B)])
    out2 = np.stack([res.results[B + b]["out"].reshape(C, 64, 64) for b in range(B)])
    return out1.astype(np.float32), out2.astype(np.float32)



# revision 5
# speedup vs baseline: 1.4783x; 1.4783x over previous
"""Trainium2 Bass kernel for dual cross-attention (CotSR) block.

Problem (hardcoded shapes): B=4, C=96, H=W=64 -> N=4096, C8=12, NC=96.
  For each direction d and batch b:
    q = wq @ x_q + bq            [12, N]
    k = wk @ x_kv + bk           [12, N]
    v = wv @ x_kv + bv           [96, N]
    S = q^T k  (S[n, m])         softmax over m (unscaled)
    O = sum_m softmax(S)[n, m] * v[:, m]
    out = wc @ (x_q + gamma * O) + bc

Sharding: 8 independent (direction, batch) units -> 8 NeuronCores.

Per-core device pipeline (v2 — bf16 matmuls, fused epilogue):
  - the output conv is folded into the V projection host-side:
    W2 = gamma*wc@wv, b2 = gamma*wc@bv, so the A@V matmul directly
    produces the final attention contribution (pre-normalization);
    Y0 = wc@x_q + bc is computed once in the projection phase.
  - all big matmuls run in bf16 (1 PE cycle/row vs 4 for fp32):
    projections (K=97 incl. bias-through-ones-row), scores (K=12),
    A@V (K=128 per m-block, 97 rows incl. ones row for the softmax
    denominator).
  - scores computed transposed (S^T[m, n]) so the A@V matmul needs no
    transposes: per n-chunk of 512 query rows, m-chunks of 3/2 PSUM
    banks ping-pong; exp on ScalarE (ACT) straight out of PSUM into
    bf16 SBUF tiles; A@V software-pipelined one m-chunk behind.
  - per-chunk epilogue (software-pipelined one n-chunk behind): copy
    O'[97,512] to SBUF, reciprocal of row 96 (=denominator) on DVE,
    rank-1 fp32r matmul broadcasts 1/l to [96,512], then DVE
    multiply + add Y0 and DMA out.
"""

import numpy as np

B, C, C8, NC = 4, 96, 12, 96
N = 4096  # 64*64
CP = 97  # C + ones row
NCH = 8  # n chunks of 512
NB = 512
MB = 32  # m blocks of 128 per n-chunk
# m-chunk sizes (in 128-row m-blocks): 3-bank / 3-bank ping-pong + tail
M_CHUNKS = [3] * 10 + [2]
assert sum(M_CHUNKS) == MB

_PROG = None


def _split_multi_waits(nc):
    """Walrus in this container rejects >1 sync wait per instruction.
    Split extra waits onto same-engine NOPs inserted just before."""
    import concourse.mybir as mybir

    n_split = 0
    for bb in nc.main_func.blocks:
        insts = list(bb.instructions)
        if not any(i.sync_info and len(i.sync_info.on_wait) > 1 for i in insts):
            continue
        new = []
        for inst in insts:
            si = inst.sync_info
            if si is not None and len(si.on_wait) > 1:
                waits = list(si.on_wait)
                for w in waits[:-1]:
                    n_split += 1
                    new.append(
                        mybir.InstNoOp(
                            name=f"{inst.name}-wsplit{n_split}",
                            engine=inst.engine,
                            ins=[],
                            outs=[],
                            sync_info=mybir.SyncInfo(on_wait=[w], on_update=[]),
                        )
                    )
                inst.sync_info = mybir.SyncInfo(
                    on_wait=[waits[-1]], on_update=list(si.on_update)
                )
            new.append(inst)
        bb.instructions = new
    return n_split


def _build_program():
    import concourse.bass as bass
    import concourse.mybir as mybir
    import concourse.tile as tile

    f32 = mybir.dt.float32
    f32r = mybir.dt.float32r
    bf16 = mybir.dt.bfloat16
    nc = bass.Bass()

    xq_d = nc.dram_tensor("xq", [CP, N], bf16, kind="ExternalInput")
    xkv_d = nc.dram_tensor("xkv", [CP, N], bf16, kind="ExternalInput")
    wqT_d = nc.dram_tensor("wqT", [CP, C8], bf16, kind="ExternalInput")
    wkT_d = nc.dram_tensor("wkT", [CP, C8], bf16, kind="ExternalInput")
    wvT_d = nc.dram_tensor("wvT", [CP, NC], bf16, kind="ExternalInput")
    wcA_d = nc.dram_tensor("wcA", [CP, NC], bf16, kind="ExternalInput")
    out_d = nc.dram_tensor("out", [NC, N], f32, kind="ExternalOutput")

    with tile.TileContext(nc) as tc:
        with tc.tile_pool(name="persist", bufs=1) as pp:
            xq = pp.tile([CP, N], bf16)
            xkv = pp.tile([CP, N], bf16)
            wqT = pp.tile([CP, C8], bf16)
            wkT = pp.tile([CP, C8], bf16)
            wvT = pp.tile([CP, NC], bf16)
            wcA = pp.tile([CP, NC], bf16)
            nc.sync.dma_start(out=xq, in_=xq_d[:])
            nc.sync.dma_start(out=xkv, in_=xkv_d[:])
            nc.sync.dma_start(out=wqT, in_=wqT_d[:])
            nc.sync.dma_start(out=wkT, in_=wkT_d[:])
            nc.sync.dma_start(out=wvT, in_=wvT_d[:])
            nc.sync.dma_start(out=wcA, in_=wcA_d[:])

            bufQ = pp.tile([C8, N], bf16)
            bufK = pp.tile([C8, N], bf16)
            VT1 = pp.tile([128, MB * CP], bf16)  # per m-block [V''^T | 1]
            Y0 = pp.tile([NC, N], f32)  # wc@x_q + bc
            bcw = pp.tile([1, NC], bf16)  # ones row for 1/l broadcast
            nc.vector.memset(VT1, 1.0)
            nc.vector.memset(bcw, 1.0)

            # ---- projections + Y0 ----
            with tc.tile_pool(name="ps_setup", bufs=1, space="PSUM") as pss:
                for c in range(NCH):
                    sl = slice(NB * c, NB * (c + 1))
                    pq = pss.tile([C8, NB], f32, tag="pqk", bufs=2)
                    nc.tensor.matmul(pq, lhsT=wqT, rhs=xq[:, sl], start=True, stop=True)
                    nc.vector.tensor_copy(bufQ[:, sl], pq)
                    pk = pss.tile([C8, NB], f32, tag="pqk", bufs=2)
                    nc.tensor.matmul(pk, lhsT=wkT, rhs=xkv[:, sl], start=True, stop=True)
                    nc.vector.tensor_copy(bufK[:, sl], pk)
                    py0 = pss.tile([NC, NB], f32, tag="py0", bufs=2)
                    nc.tensor.matmul(py0, lhsT=wcA, rhs=xq[:, sl], start=True, stop=True)
                    nc.vector.tensor_copy(Y0[:, sl], py0)
                for mb in range(MB):
                    pv = pss.tile([128, NC], f32, tag="pv", bufs=4)
                    nc.tensor.matmul(
                        pv,
                        lhsT=xkv[:, 128 * mb : 128 * (mb + 1)],
                        rhs=wvT,
                        start=True,
                        stop=True,
                    )
                    nc.vector.tensor_copy(VT1[:, CP * mb : CP * mb + NC], pv)

            # ---- main loop: scores -> exp -> A@V, epilogue pipelined ----
            with (
                tc.tile_pool(name="ps_main", bufs=1, space="PSUM") as psm,
                tc.tile_pool(name="epool", bufs=1) as ep,
            ):
                epi_pend = None  # ps_o of previous n-chunk awaiting epilogue

                def emit_epilogue(ps_o, c):
                    nsl = slice(NB * c, NB * (c + 1))
                    oun = ep.tile([CP, NB], f32, tag="oun", bufs=2)
                    nc.vector.tensor_copy(oun, ps_o)
                    rr = ep.tile([1, NB], bf16, tag="rr", bufs=2)
                    with nc.allow_low_precision(
                        reason="bf16 1/l adds ~0.2% noise on the attention term only"
                    ):
                        nc.vector.reciprocal(out=rr, in_=oun[NC : NC + 1, :])
                    R = psm.tile([NC, NB], f32, tag="R", bufs=1)
                    nc.tensor.matmul(R, lhsT=bcw, rhs=rr, start=True, stop=True)
                    yb = ep.tile([NC, NB], f32, tag="yb", bufs=2)
                    nc.vector.tensor_mul(out=yb, in0=oun[0:NC, :], in1=R)
                    nc.vector.tensor_add(out=yb, in0=yb, in1=Y0[:, nsl])
                    nc.sync.dma_start(out=out_d[:, nsl], in_=yb)

                for c in range(NCH):
                    nsl = slice(NB * c, NB * (c + 1))
                    ps_o = psm.tile([CP, NB], f32, tag="ps_o", bufs=1)
                    pending = None  # (e_tile, mb0, nblk)
                    mb0 = 0
                    for t, nblk in enumerate(M_CHUNKS):
                        sc = psm.tile([128, 3 * NB], f32, tag=f"sc{t % 2}", bufs=1)
                        for s in range(nblk):
                            m0 = 128 * (mb0 + s)
                            nc.tensor.matmul(
                                sc[:, NB * s : NB * (s + 1)],
                                lhsT=bufK[:, m0 : m0 + 128],
                                rhs=bufQ[:, nsl],
                                start=True,
                                stop=True,
                            )
                        # previous n-chunk's epilogue: emitted after the first
                        # score group so its tiny PE matmul never stalls PE
                        nonlocal_epi = epi_pend if t == 0 else None
                        if nonlocal_epi is not None:
                            emit_epilogue(nonlocal_epi, c - 1)
                        # software pipeline: previous m-chunk's AV after this
                        # chunk's score matmuls so PE overlaps ACT exp
                        if pending is not None:
                            pe, pmb0, pnblk = pending
                            for s in range(pnblk):
                                mb = pmb0 + s
                                nc.tensor.matmul(
                                    ps_o,
                                    lhsT=VT1[:, CP * mb : CP * (mb + 1)],
                                    rhs=pe[:, NB * s : NB * (s + 1)],
                                    start=(mb == 0),
                                    stop=False,
                                )
                        e = ep.tile([128, 3 * NB], bf16, tag="e", bufs=3)
                        nc.scalar.activation(
                            out=e[:, : NB * nblk],
                            in_=sc[:, : NB * nblk],
                            func=mybir.ActivationFunctionType.Exp,
                        )
                        pending = (e, mb0, nblk)
                        mb0 += nblk
                    pe, pmb0, pnblk = pending
                    for s in range(pnblk):
                        mb = pmb0 + s
                        nc.tensor.matmul(
                            ps_o,
                            lhsT=VT1[:, CP * mb : CP * (mb + 1)],
                            rhs=pe[:, NB * s : NB * (s + 1)],
                            start=False,
                            stop=(mb == MB - 1),
                        )
                    epi_pend = ps_o
                emit_epilogue(epi_pend, NCH - 1)

    _split_multi_waits(nc)
    return nc


def _get_program():
    global _PROG
    if _PROG is None:
        _PROG = _build_program()
    return _PROG


TRACE = False
LAST_RESULT = None


def _to_bf16(a):
    import ml_dtypes

    return np.asarray(a, np.float32).astype(ml_dtypes.bfloat16)


def _pack_weights(wq, bq, wk, bk, wv, bv, gamma, wc, bc):
    g = float(np.asarray(gamma).reshape(-1)[0])
    wqT = np.zeros((CP, C8), np.float32)
    wqT[0:C, :] = wq.T
    wqT[C, :] = bq
    wkT = np.zeros((CP, C8), np.float32)
    wkT[0:C, :] = wk.T
    wkT[C, :] = bk
    # fold gamma*wc into the V projection
    w2 = g * (wc @ wv)  # [NC, C]
    b2 = g * (wc @ bv)  # [NC]
    wvT = np.zeros((CP, NC), np.float32)
    wvT[0:C, :] = w2.T
    wvT[C, :] = b2
    wcA = np.zeros((CP, NC), np.float32)
    wcA[0:C, :] = wc.T
    wcA[C, :] = bc
    return tuple(_to_bf16(a) for a in (wqT, wkT, wvT, wcA))


def _pack_x(x):
    buf = np.empty((CP, N), np.float32)
    buf[0:C, :] = x.reshape(C, N)
    buf[C, :] = 1.0
    return _to_bf16(buf)


def kernel(x1, x2, wq1, bq1, wk1, bk1, wv1, bv1, wq2, bq2, wk2, bk2,
           wv2, bv2, gamma1, gamma2, wc1, bc1, wc2, bc2):
    from concourse.bass_utils import run_bass_kernel_spmd

    global LAST_RESULT
    x1 = np.asarray(x1, np.float32)
    x2 = np.asarray(x2, np.float32)

    w1 = _pack_weights(
        np.asarray(wq1), np.asarray(bq1), np.asarray(wk2), np.asarray(bk2),
        np.asarray(wv2), np.asarray(bv2), np.asarray(gamma1),
        np.asarray(wc1), np.asarray(bc1),
    )
    w2 = _pack_weights(
        np.asarray(wq2), np.asarray(bq2), np.asarray(wk1), np.asarray(bk1),
        np.asarray(wv1), np.asarray(bv1), np.asarray(gamma2),
        np.asarray(wc2), np.asarray(bc2),
    )

    in_maps = []
    for d in range(2):
        xs_q, xs_kv = (x1, x2) if d == 0 else (x2, x1)
        wqT, wkT, wvT, wcA = w1 if d == 0 else w2
        for b in range(B):
            in_maps.append(
                {
                    "xq": _pack_x(xs_q[b]),
                    "xkv": _pack_x(xs_kv[b]),
                    "wqT": wqT,
                    "wkT": wkT,
                    "wvT": wvT,
                    "wcA": wcA,
                }
            )

    nc = _get_program()
    res = run_bass_kernel_spmd(nc, in_maps, core_ids=list(range(8)), trace=TRACE)
    LAST_RESULT = res

    out1 = np.stack([res.results[b]["out"].reshape(C, 64, 64) for b in range(B)])
    out2 = np.stack([res.results[B + b]["out"].reshape(C, 64, 64) for b in range(B)])
    return out1.astype(np.float32), out2.astype(np.float32)
